# revision 29
# baseline (speedup 1.0000x reference)
"""TRN2 Bass kernel for nn_Attention_39316130628152.

Spatial self-attention: B=4, C=64, H=W=64 (N=4096 tokens), f32.
  q/k/v = 1x1conv(x);  out = v @ softmax(q^T k)^T

Sharding: 8 cores = (batch b in 0..3) x (query-half h in 0..1).
Each core handles 2048 queries x 4096 keys for one batch.

Per-core algorithm:
  q[c,i], k[c,j]: fp16 projections, bias folded via ones-row of x_aug.
  q/k are materialized twice (partitions 0-63 and 64-127, via col-tiled
  projection matmuls) so scores for TWO j-tiles run CONCURRENTLY in the
  PE array as row-tiled K=64 matmuls (tile_position (0,0) / (64,0)).
  for each i-macro (512 queries), j-pair (2x128 keys):
    sT[j,i] = k_tile^T q        (fp16 row-tiled pair -> fp32 PSUM 2 banks)
    p = exp(sT - 40)            (one ACT instr over both banks, bf16 out)
    U[e,i] += x_aug[e,:] p      (bf16, PSUM accum; ones-row => U[64]=Z)
  outT[i,c] = U_aug^T WvT2      (fp32r; WvT2=[[Wv^T,0],[bv,1]] => col 64=Z)
  out_norm[i,c] = outT[i,c] / Z[i]   (DVE reciprocal + per-partition scale)

Output per core: [16, 128, 64] tiles of out_norm[i, c]; host reassembles.
"""
import numpy as np
import ml_dtypes

import concourse.bacc as bacc
import concourse.mybir as mybir
import concourse.tile as tile
from concourse.bass_utils import run_bass_kernel_spmd

F32 = mybir.dt.float32
F32R = mybir.dt.float32r
F16 = mybir.dt.float16
BF16 = mybir.dt.bfloat16

B, C, HH, WW = 4, 64, 64, 64
N = HH * WW           # 4096 tokens
NQ = N // 2           # queries per core (2048)
IM = 512              # i-macro size
NIM = NQ // IM        # 4
JT = 128              # j-tile (keys per tile)
NPAIR = N // (2 * JT)  # 16 j-pairs
NCH = IM // 128       # output chunks per i-macro (4)
EXP_BIAS = -40.0      # exp(s + EXP_BIAS); cancels in normalization

_NC_CACHE = {}


def build_nc():
    if "nc" in _NC_CACHE:
        return _NC_CACHE["nc"]
    nc = bacc.Bacc(None, target_bir_lowering=False)

    XA = nc.dram_tensor("XA", (C + 1, N), F16, kind="ExternalInput")
    XQ = nc.dram_tensor("XQ", (C + 1, NQ), F16, kind="ExternalInput")
    XT = nc.dram_tensor("XT", (128, N // JT, C + 1), BF16, kind="ExternalInput")
    WQT = nc.dram_tensor("WQT", (C + 1, C), F16, kind="ExternalInput")
    WKT = nc.dram_tensor("WKT", (C + 1, C), F16, kind="ExternalInput")
    WVT2 = nc.dram_tensor("WVT2", (C + 1, C + 2), F32R, kind="ExternalInput")
    OUT = nc.dram_tensor("OUT", (NIM * NCH, 128, C), F32, kind="ExternalOutput")

    with tile.TileContext(nc) as tc:
        with (
            tc.tile_pool(name="consts", bufs=1) as consts,
            tc.tile_pool(name="acts", bufs=1) as acts,
            tc.tile_pool(name="pexp", bufs=3) as pexp,
            tc.tile_pool(name="usbp", bufs=2) as usbp,
            tc.tile_pool(name="small", bufs=4) as small,
            tc.tile_pool(name="resp", bufs=4) as resp,
            tc.tile_pool(name="psS", bufs=3, space="PSUM") as psS,
            tc.tile_pool(name="psU", bufs=2, space="PSUM") as psU,
        ):
            ebias_sb = consts.tile([128, 1], F32, tag="ebias")
            nc.vector.memset(ebias_sb, EXP_BIAS)
            # dummy exp: pulls the ~2.7us ACT table load into the DMA head
            dume_sb = consts.tile([128, 2], F32, tag="dume")
            nc.scalar.activation(dume_sb[:, 0:1], ebias_sb[:, :],
                                 mybir.ActivationFunctionType.Exp)
            # Per-chunk tiles: Tile tracks deps at tile granularity, so
            # chunked tiles let the first scores pair start after ONE
            # DMA + projection + evac chain instead of all of them.
            wq_sb = consts.tile([C + 1, C], F16, tag="wq")
            wk_sb = consts.tile([C + 1, C], F16, tag="wk")
            wv2_sb = consts.tile([C + 1, C + 2], F32R, tag="wv2")
            xa_sb = [consts.tile([C + 1, 512], F16, tag=f"xa{t}", name=f"xa{t}")
                     for t in range(8)]
            xq_sb = [consts.tile([C + 1, 512], F16, tag=f"xq{t}", name=f"xq{t}")
                     for t in range(4)]
            xt_sb = [consts.tile([128, 16, C + 1], BF16, tag=f"xt{t}", name=f"xt{t}")
                     for t in range(2)]
            # spread input DMAs over the two HWDGE rings (sync, scalar) and
            # the gpsimd SWDGE ring so the pipeline fill isn't DMA-serial
            # ring heads: the tensors the first projections need
            nc.sync.dma_start(out=xa_sb[0], in_=XA[:, 0:512])
            nc.scalar.dma_start(out=xq_sb[0], in_=XQ[:, 0:512])
            nc.sync.dma_start(out=wk_sb, in_=WKT[:, :])
            nc.scalar.dma_start(out=wq_sb, in_=WQT[:, :])
            nc.sync.dma_start(out=xa_sb[1], in_=XA[:, 512:1024])
            for t in range(2, 8):
                (nc.scalar if t % 2 else nc.sync).dma_start(
                    out=xa_sb[t], in_=XA[:, t * 512:(t + 1) * 512])
            for t in range(1, 4):
                (nc.sync if t % 2 else nc.scalar).dma_start(
                    out=xq_sb[t], in_=XQ[:, t * 512:(t + 1) * 512])
            for t in range(2):
                nc.gpsimd.dma_start(out=xt_sb[t],
                                    in_=XT[:, t * 16:(t + 1) * 16, :])
            nc.gpsimd.dma_start(out=wv2_sb, in_=WVT2[:, :])

            # Projections (fp16, K=65: bias rides the ones-row).  Each
            # 512-wide tile is produced TWICE via col-tiled matmuls
            # (array cols 0-63 -> out partitions 0-63, cols 64-127 ->
            # 64-127) so scores can row-tile over duplicated q/k.
            def project(dst, w_sb, src):
                ps = psS.tile([128, 1024], F32, tag="s", name="proj_ps")
                nc.tensor.matmul(ps[0:C, 0:512], w_sb[:, :], src[:, :],
                                 start=True, stop=True, tile_position=(0, 0))
                nc.tensor.matmul(ps[C:2 * C, 0:512], w_sb[:, :], src[:, :],
                                 start=True, stop=True, tile_position=(0, 64))
                nc.vector.tensor_copy(dst[:, :], ps[:, 0:512])

            # k chunk 0 and q chunk 0 first: the main loop's first score
            # pairs depend only on them.
            q_sb = [acts.tile([2 * C, 512], F16, tag=f"q{t}", name=f"q{t}")
                    for t in range(4)]
            k_sb = [acts.tile([2 * C, 512], F16, tag=f"k{t}", name=f"k{t}")
                    for t in range(8)]
            project(k_sb[0], wk_sb, xa_sb[0])
            project(q_sb[0], wq_sb, xq_sb[0])
            for t in range(1, 8):
                project(k_sb[t], wk_sb, xa_sb[t])
            for t in range(1, 4):
                project(q_sb[t], wq_sb, xq_sb[t])

            def epilogue(im, u_sb):
                for ch in range(NCH):
                    o_ps = psU.tile([128, C + 2], F32, tag="u")
                    nc.tensor.matmul(o_ps[:, :],
                                     u_sb[:, ch * 128:(ch + 1) * 128],
                                     wv2_sb[:, :], start=True, stop=True)
                    r_sb = small.tile([128, 1], F32, tag="r")
                    nc.vector.reciprocal(r_sb[:, :], o_ps[:, C:C + 1])
                    res_sb = resp.tile([128, C], F32, tag="res")
                    nc.vector.tensor_scalar_mul(res_sb[:, :], o_ps[:, 0:C],
                                                r_sb[:, :])
                    nc.sync.dma_start(out=OUT[im * NCH + ch, :, :], in_=res_sb)

            pending = None  # software-pipelined epilogue of the previous im
            for im in range(NIM):
                u_ps = psU.tile([C + 1, IM], F32, tag="u")
                qc = q_sb[im]
                for t in range(NPAIR):
                    jtA, jtB = 2 * t, 2 * t + 1
                    kc = k_sb[t // 2]
                    ko = (t % 2) * 256
                    s_ps = psS.tile([128, 1024], F32, tag="s")
                    nc.tensor.matmul(
                        s_ps[:, 0:512],
                        kc[0:C, ko:ko + JT],
                        qc[0:C, :],
                        start=True, stop=True, tile_position=(0, 0))
                    nc.tensor.matmul(
                        s_ps[:, 512:1024],
                        kc[C:2 * C, ko + JT:ko + 2 * JT],
                        qc[C:2 * C, :],
                        start=True, stop=True, tile_position=(64, 0))
                    p_sb = pexp.tile([128, 1024], BF16, tag="p")
                    nc.scalar.activation(p_sb[:, :], s_ps[:, :],
                                         mybir.ActivationFunctionType.Exp,
                                         bias=ebias_sb[:, :])
                    nc.tensor.matmul(
                        u_ps[:, :], xt_sb[jtA // 16][:, jtA % 16, :],
                        p_sb[:, 0:512],
                        start=(t == 0), stop=False)
                    nc.tensor.matmul(
                        u_ps[:, :], xt_sb[jtB // 16][:, jtB % 16, :],
                        p_sb[:, 512:1024],
                        start=False, stop=(t == NPAIR - 1))
                    if t == 1 and pending is not None:
                        epilogue(im - 1, pending)
                        pending = None
                u_sb = usbp.tile([C + 1, IM], F32R, tag="u_sb")
                nc.vector.tensor_copy(u_sb[:, :], u_ps[:, :])
                pending = u_sb
            epilogue(NIM - 1, pending)
    nc.finalize()
    _NC_CACHE["nc"] = nc
    return nc


def prep_inputs(x, Wq, bq, Wk, bk, Wv, bv):
    """Build the 8 per-core input maps (host-side numpy, cheap)."""
    f32 = np.float32
    wqt = np.concatenate([Wq.T, bq[None, :]], axis=0).astype(np.float16)
    wkt = np.concatenate([Wk.T, bk[None, :]], axis=0).astype(np.float16)
    wvt2 = np.zeros((C + 1, C + 2), dtype=f32)  # col C+1 = pad (even free)
    wvt2[:C, :C] = Wv.T
    wvt2[C, :C] = bv
    wvt2[C, C] = 1.0

    in_maps = []
    for core in range(8):
        b, h = core // 2, core % 2
        xb = np.ascontiguousarray(x[b].reshape(C, N)).astype(f32)
        xa = np.concatenate([xb, np.ones((1, N), dtype=f32)], axis=0)
        xa16 = xa.astype(np.float16)
        xq16 = np.ascontiguousarray(xa16[:, h * NQ:(h + 1) * NQ])
        # XT[p, jt, e] = xa[e, jt*128 + p], bf16
        xt = np.ascontiguousarray(
            xa.T.reshape(N // JT, 128, C + 1).transpose(1, 0, 2)
        ).astype(ml_dtypes.bfloat16)
        in_maps.append(dict(XA=xa16, XQ=xq16, XT=xt,
                            WQT=wqt, WKT=wkt, WVT2=wvt2))
    return in_maps


def assemble_output(results):
    out = np.empty((B, C, N), dtype=np.float32)
    for core in range(8):
        b, h = core // 2, core % 2
        o = results[core]["OUT"].reshape(NQ, C)  # [i, c]
        out[b, :, h * NQ:(h + 1) * NQ] = o.T
    return out.reshape(B, C, HH, WW)


def kernel(x, Wq, bq, Wk, bk, Wv, bv, **run_kwargs):
    x = np.asarray(x, dtype=np.float32)
    nc = build_nc()
    in_maps = prep_inputs(np.asarray(x), np.asarray(Wq), np.asarray(bq),
                          np.asarray(Wk), np.asarray(bk),
                          np.asarray(Wv), np.asarray(bv))
    res = run_bass_kernel_spmd(nc, in_maps, core_ids=list(range(8)),
                               **run_kwargs)
    out = assemble_output(res.results)
    if run_kwargs:
        return out, res
    return out


if __name__ == "__main__":
    rng = np.random.default_rng(0)
    s = 1.0 / np.sqrt(C)
    x = rng.standard_normal((B, C, HH, WW), dtype=np.float32)
    args = dict(
        x=x,
        Wq=(rng.standard_normal((C, C), dtype=np.float32) * s),
        bq=(rng.standard_normal(C, dtype=np.float32) * 0.01),
        Wk=(rng.standard_normal((C, C), dtype=np.float32) * s),
        bk=(rng.standard_normal(C, dtype=np.float32) * 0.01),
        Wv=(rng.standard_normal((C, C), dtype=np.float32) * s),
        bv=(rng.standard_normal(C, dtype=np.float32) * 0.01),
    )
    out = kernel(**args)
    print("kernel output:", out.shape, out.dtype)
